# revision 1
# baseline (speedup 1.0000x reference)
"""AdaConv Trainium2 kernel — 8-core SPMD, data-parallel over batch.

Per core c (sample c):
  Stage A: kernel-prediction net for ALL 8 samples, layer-2 weights sharded
           by output channel across cores; AllToAll redistributes so each
           core ends with the full dynamic weights for its own sample.
  Stage B: build fused per-sample conv weights W_eff = PK o D and scatter
           them into block-diagonal stationary matrices S via diagonal-AP
           SBUF->SBUF DMA.
  Stage C: main grouped conv (128 groups of 4->4 ch, 3x3, reflect pad) as
           9 PSUM-accumulated float32r matmuls per 128-channel chunk.
"""
import sys
import types

sys.path.insert(0, "/opt/trn_rl_repo")

import numpy as np

import concourse.bass as bass
import concourse.mybir as mybir

N = 8          # batch == cores
CIN = 512
COUT = 512
HW = 64        # spatial
HWP = 66       # padded
NPOS = 16      # style spatial 4x4
OSL = 2048 // N      # dw2 out-channel slice per core (256)
PKSL = 2048 // N     # pk2 slice (256)
PBSL = 512 // N      # pb2 slice (64)
# AllToAll per-rank block: [dw 256*9 | pk 256 | pb 64]
BDW = 0
BPK = OSL * 9              # 2304
BPB = BPK + PKSL           # 2560
BLK = BPB + PBSL           # 2624
AG_SZ = N * BLK            # 20992

F32 = mybir.dt.float32
F32R = mybir.dt.float32r
BF16 = mybir.dt.bfloat16


# ---------------------------------------------------------------- tile patch
def _install_tile_patch():
    """walrus here rejects Drain instructions with >1 sync-wait; spread the
    Tile tail-drain waits over individual SP nops."""
    import concourse.tile as tile_mod
    from concourse.vector_clock import ScopedClock

    def _patched(self, tick_clock, wait_clock):
        nc = self.nc
        drain_inst = nc.sync.drain()
        wait_clock.add_sem_waits(
            drain_inst.ins, ScopedClock({None: tick_clock.global_clock})
        )
        waits = list(drain_inst.ins.sync_info.on_wait or [])
        if len(waits) > 1:
            drain_inst.ins.sync_info.on_wait = waits[:1]
            for w in waits[1:]:
                nop = nc.sync.nop(nofuse=True, hint="tail_wait_split")
                if nop.ins.sync_info is None:
                    nop.ins.sync_info = mybir.SyncInfo(on_wait=[w], on_update=[])
                else:
                    nop.ins.sync_info.on_wait = [w]
        nc.all_engine_barrier()
        assert self.sems is not None
        popped = nc._tile_sem_poison_stack.pop()
        assert popped is self._sem_poison
        nc.clear_and_free_semaphores(list(self.sems.allocated().values()))
        nc.all_engine_barrier()

    tile_mod.TileContext._drain_and_barrier = _patched


_install_tile_patch()
from concourse.tile import TileContext  # noqa: E402


def install_profile_shim():
    """antenv.axon_hooks is missing from this image; recreate it so
    run_bass_kernel_spmd(trace=True) can capture NTFF profiles."""
    if "antenv.axon_hooks" in sys.modules:
        return
    import antenv

    mod = types.ModuleType("antenv.axon_hooks")
    mod._hook = None
    mod.set_axon_ntff_profile_hook = lambda h: setattr(mod, "_hook", h)
    mod.get_axon_ntff_profile_hook = lambda: mod._hook
    sys.modules["antenv.axon_hooks"] = mod
    antenv.axon_hooks = mod
    try:
        if "/root/.axon_site" not in sys.path:
            sys.path.insert(0, "/root/.axon_site")
        from trn_agent_boot.trn_boot import _ntff_profile_via_ctypes

        hook = _ntff_profile_via_ctypes("/opt/axon/libaxon_pjrt.so")
        mod.set_axon_ntff_profile_hook(hook)
    except Exception:
        pass


def _ap(t_ap, offset, dims):
    """Custom flat AP over a tile's underlying tensor."""
    return bass.AP(t_ap.tensor, offset, [list(d) for d in dims])


def _pt(t):
    """Physical partition pitch (elements) of a tile."""
    return t[:, :].ap[0][0]


def _split_excess_waits(nc, max_waits=1):
    """This walrus build rejects instructions carrying more than ~1 sync-wait.
    Move excess waits onto same-engine NoOps inserted just before."""
    n_split = 0
    for f in nc.m.functions:
        for bb in f.blocks:
            newlist = []
            for inst in bb.instructions:
                si = getattr(inst, "sync_info", None)
                if si is not None and si.on_wait and len(si.on_wait) > max_waits:
                    waits = list(si.on_wait)
                    for k, w in enumerate(waits[max_waits:]):
                        nop = mybir.InstNoOp(
                            name=f"{inst.name}_ws{k}",
                            engine=inst.engine,
                            bass_nofuse=True,
                            sync_info=mybir.SyncInfo(on_wait=[w], on_update=[]),
                        )
                        newlist.append(nop)
                        n_split += 1
                    si.on_wait = waits[:max_waits]
                newlist.append(inst)
            try:
                bb.instructions[:] = newlist
            except TypeError:
                bb.set_instructions(newlist)
    return n_split


def build_nc():
    nc = bass.Bass(target_bir_lowering=False)

    style = nc.declare_dram_parameter("style_all", [CIN, N * NPOS], F32, isOutput=False)
    xin = nc.declare_dram_parameter("xin", [CIN, HW * HW], F32, isOutput=False)
    w1t = nc.declare_dram_parameter("w1t", [CIN, CIN], F32, isOutput=False)
    pk1t = nc.declare_dram_parameter("pk1t", [CIN, CIN], F32, isOutput=False)
    pb1t = nc.declare_dram_parameter("pb1t", [CIN, CIN], F32, isOutput=False)
    w2t = nc.declare_dram_parameter("w2t", [16, 128, OSL], F32, isOutput=False)
    pk2t = nc.declare_dram_parameter("pk2t", [4, 128, PKSL], F32, isOutput=False)
    pb2t = nc.declare_dram_parameter("pb2t", [4, 128, PBSL], F32, isOutput=False)
    b1 = nc.declare_dram_parameter("b1", [CIN], F32, isOutput=False)
    bk1 = nc.declare_dram_parameter("bk1", [CIN], F32, isOutput=False)
    bb1 = nc.declare_dram_parameter("bb1", [CIN], F32, isOutput=False)
    b2s = nc.declare_dram_parameter("b2s", [OSL], F32, isOutput=False)
    bk2s = nc.declare_dram_parameter("bk2s", [PKSL], F32, isOutput=False)
    bb2s = nc.declare_dram_parameter("bb2s", [PBSL], F32, isOutput=False)
    perm = nc.declare_dram_parameter("perm", [4, 128, 128], F32, isOutput=False)
    ident = nc.declare_dram_parameter("ident", [128, 128], F32, isOutput=False)
    selm = nc.declare_dram_parameter("selm", [36, 9 * 128], F32, isOutput=False)
    maskm = nc.declare_dram_parameter("maskm", [128, 128], F32, isOutput=False)
    out = nc.declare_dram_parameter("out", [COUT, HW * HW], F32, isOutput=True)

    with TileContext(nc) as tc:
        with (
            tc.tile_pool(name="sb", bufs=1) as sb,
            tc.tile_pool(name="sbx", bufs=1) as sbx,
            tc.tile_pool(name="sbo", bufs=2) as sbo,
            tc.tile_pool(name="ps", bufs=2, space="PSUM") as ps,
            tc.tile_pool(name="psc", bufs=4, space="PSUM") as psc,
            tc.tile_pool(name="dram", bufs=1, space="DRAM") as dram,
        ):
            # ---------------- stage C input loads first (prefetch)
            xp = [sbx.tile([128, 4384], F32, tag=f"xp{ch}", name=f"xp{ch}") for ch in range(4)]
            for ch in range(4):
                dst = _ap(xp[ch], HWP + 1, [[4384, 128], [HWP, HW], [1, HW]])
                nc.sync.dma_start(out=dst, in_=xin[ch * 128:(ch + 1) * 128, :])

            # ---------------- stage A: layer-1 (h = lrelu(W1 s + b1))
            st = []
            for i in range(4):
                t = sb.tile([128, N * NPOS], F32, tag=f"st{i}", name=f"st{i}")
                nc.sync.dma_start(out=t[:, :], in_=style[i * 128:(i + 1) * 128, :])
                st.append(t)
            w1sb = sb.tile([128, 4 * CIN], F32, tag="w1sb", name="w1sb")
            for it in range(4):
                nc.sync.dma_start(
                    out=_ap(w1sb, it * CIN, [[4 * CIN, 128], [1, CIN]]),
                    in_=w1t[it * 128:(it + 1) * 128, :],
                )
            b1t = sb.tile([128, 4], F32, tag="b1t", name="b1t")
            for ot in range(4):
                nc.sync.dma_start(
                    out=b1t[:, ot:ot + 1], in_=_ap(b1[:], ot * 128, [[1, 128], [1, 1]])
                )
            h = [sb.tile([128, N * NPOS], F32, tag=f"h{ot}", name=f"h{ot}") for ot in range(4)]
            for ot in range(4):
                hp = ps.tile([128, N * NPOS], F32, tag="sA", name="sA")
                for it in range(4):
                    nc.tensor.matmul(
                        hp[:, :],
                        _ap(w1sb, it * CIN + ot * 128, [[4 * CIN, 128], [1, 128]]),
                        st[it][:, :],
                        start=(it == 0),
                        stop=(it == 3),
                    )
                nc.scalar.activation(
                    h[ot][:, :], hp[:, :], mybir.ActivationFunctionType.Identity,
                    bias=b1t[:, ot:ot + 1],
                )
                zt = sb.tile([128, N * NPOS], F32, tag="zt", name="zt")
                nc.vector.tensor_scalar_mul(zt[:, :], h[ot][:, :], 0.01)
                nc.vector.tensor_max(h[ot][:, :], h[ot][:, :], zt[:, :])

            # ---------------- stage A: dw2 slice for all samples
            w2sb = sb.tile([128, 16 * OSL], F32, tag="w2sb", name="w2sb")
            nc.sync.dma_start(
                out=_ap(w2sb, 0, [[16 * OSL, 128], [OSL, 16], [1, OSL]]),
                in_=_ap(w2t[:, :, :], 0, [[OSL, 128], [128 * OSL, 16], [1, OSL]]),
            )
            b2t = sb.tile([128, 2], F32, tag="b2t", name="b2t")
            for o2 in range(2):
                nc.sync.dma_start(
                    out=b2t[:, o2:o2 + 1],
                    in_=_ap(b2s[:], o2 * 128, [[1, 128], [1, 1]]),
                )
            agin = dram.tile([AG_SZ], F32)
            agout = dram.tile([AG_SZ], F32)
            dwc = [sb.tile([128, 96], F32, tag=f"dwc{o2}", name=f"dwc{o2}") for o2 in range(2)]
            for o2 in range(2):
                dps = ps.tile([128, N * 9], F32, tag="sA", name="sA")
                for kt in range(16):
                    it, tap = kt // 4, kt % 4
                    di, dj = tap // 2, tap % 2
                    rhs = _ap(h[it], di * 4 + dj,
                              [[N * NPOS, 128], [NPOS, N], [4, 3], [1, 3]])
                    nc.tensor.matmul(
                        dps[:, :],
                        _ap(w2sb, kt * OSL + o2 * 128, [[16 * OSL, 128], [1, 128]]),
                        rhs,
                        start=(kt == 0),
                        stop=(kt == 15),
                    )
                nc.scalar.activation(
                    dwc[o2][:, 0:72], dps[:, :], mybir.ActivationFunctionType.Identity,
                    bias=b2t[:, o2:o2 + 1],
                )
                # agin[n*BLK + (o2*128+p)*9 + pos] = dwc[o2][p, n*9+pos]
                nc.sync.dma_start(
                    out=_ap(agin[:], o2 * 128 * 9, [[9, 128], [BLK, N], [1, 9]]),
                    in_=_ap(dwc[o2], 0, [[96, 128], [9, N], [1, 9]]),
                )

            # ---------------- stage A: pooled-style path (pk / pb)
            sp = [sb.tile([128, N], F32, tag=f"sp{i}", name=f"sp{i}") for i in range(4)]
            for i in range(4):
                nc.vector.tensor_reduce(
                    sp[i][:, :],
                    _ap(st[i], 0, [[N * NPOS, 128], [NPOS, N], [1, NPOS]]),
                    axis=mybir.AxisListType.X,
                    op=mybir.AluOpType.add,
                )
                nc.vector.tensor_scalar_mul(sp[i][:, :], sp[i][:, :], 1.0 / NPOS)

            def layer1(wt_param, bias_param, tagp):
                wsb = sb.tile([128, 4 * CIN], F32, tag=f"{tagp}w", name=f"{tagp}w")
                for it in range(4):
                    nc.sync.dma_start(
                        out=_ap(wsb, it * CIN, [[4 * CIN, 128], [1, CIN]]),
                        in_=wt_param[it * 128:(it + 1) * 128, :],
                    )
                bt = sb.tile([128, 4], F32, tag=f"{tagp}b", name=f"{tagp}b")
                for ot in range(4):
                    nc.sync.dma_start(
                        out=bt[:, ot:ot + 1],
                        in_=_ap(bias_param[:], ot * 128, [[1, 128], [1, 1]]),
                    )
                acts = []
                for ot in range(4):
                    ap_ = ps.tile([128, N], F32, tag="sA", name="sA")
                    for it in range(4):
                        nc.tensor.matmul(
                            ap_[:, :],
                            _ap(wsb, it * CIN + ot * 128, [[4 * CIN, 128], [1, 128]]),
                            sp[it][:, :],
                            start=(it == 0),
                            stop=(it == 3),
                        )
                    a = sb.tile([128, N], F32, tag=f"{tagp}a{ot}", name=f"{tagp}a{ot}")
                    nc.scalar.activation(
                        a[:, :], ap_[:, :], mybir.ActivationFunctionType.Identity,
                        bias=bt[:, ot:ot + 1],
                    )
                    zt2 = sb.tile([128, N], F32, tag="zt2", name="zt2")
                    nc.vector.tensor_scalar_mul(zt2[:, :], a[:, :], 0.01)
                    nc.vector.tensor_max(a[:, :], a[:, :], zt2[:, :])
                    acts.append(a)
                return acts

            a1 = layer1(pk1t, bk1, "pk1")
            c1 = layer1(pb1t, bb1, "pb1")

            pk2sb = sb.tile([128, 4 * PKSL], F32, tag="pk2sb", name="pk2sb")
            nc.sync.dma_start(
                out=_ap(pk2sb, 0, [[4 * PKSL, 128], [PKSL, 4], [1, PKSL]]),
                in_=_ap(pk2t[:, :, :], 0, [[PKSL, 128], [128 * PKSL, 4], [1, PKSL]]),
            )
            bk2t = sb.tile([128, 2], F32, tag="bk2t", name="bk2t")
            for o2 in range(2):
                nc.sync.dma_start(
                    out=bk2t[:, o2:o2 + 1],
                    in_=_ap(bk2s[:], o2 * 128, [[1, 128], [1, 1]]),
                )
            for o2 in range(2):
                pp = ps.tile([128, N], F32, tag="sA", name="sA")
                for it in range(4):
                    nc.tensor.matmul(
                        pp[:, :],
                        _ap(pk2sb, it * PKSL + o2 * 128, [[4 * PKSL, 128], [1, 128]]),
                        a1[it][:, :],
                        start=(it == 0),
                        stop=(it == 3),
                    )
                pkc = sb.tile([128, 32], F32, tag=f"pkc{o2}", name=f"pkc{o2}")
                nc.scalar.activation(
                    pkc[:, 0:8], pp[:, :], mybir.ActivationFunctionType.Identity,
                    bias=bk2t[:, o2:o2 + 1],
                )
                nc.sync.dma_start(
                    out=_ap(agin[:], BPK + o2 * 128, [[1, 128], [BLK, N]]),
                    in_=_ap(pkc, 0, [[32, 128], [1, N]]),
                )

            pb2sb = sb.tile([128, 4 * PBSL], F32, tag="pb2sb", name="pb2sb")
            nc.sync.dma_start(
                out=_ap(pb2sb, 0, [[4 * PBSL, 128], [PBSL, 4], [1, PBSL]]),
                in_=_ap(pb2t[:, :, :], 0, [[PBSL, 128], [128 * PBSL, 4], [1, PBSL]]),
            )
            bb2t = sb.tile([64, 1], F32, tag="bb2t", name="bb2t")
            nc.sync.dma_start(
                out=bb2t[:, 0:1], in_=_ap(bb2s[:], 0, [[1, 64], [1, 1]])
            )
            pbp = ps.tile([64, N], F32, tag="sA", name="sA")
            for it in range(4):
                nc.tensor.matmul(
                    pbp[:, :],
                    _ap(pb2sb, it * PBSL, [[4 * PBSL, 128], [1, PBSL]]),
                    c1[it][:, :],
                    start=(it == 0),
                    stop=(it == 3),
                )
            pbc = sb.tile([64, 32], F32, tag="pbc", name="pbc")
            nc.scalar.activation(
                pbc[:, 0:8], pbp[:, :], mybir.ActivationFunctionType.Identity,
                bias=bb2t[:, 0:1],
            )
            nc.sync.dma_start(
                out=_ap(agin[:], BPB, [[1, 64], [BLK, N]]),
                in_=_ap(pbc, 0, [[32, 64], [1, N]]),
            )

            # ---------------- AllToAll: core c receives, from every rank r,
            # rank r's o-slice of sample c's dynamic weights.
            nc.gpsimd.collective_compute(
                "AllToAll",
                mybir.AluOpType.bypass,
                replica_groups=[list(range(N))],
                ins=[agin[:].opt()],
                outs=[agout[:].opt()],
            )

            # ---------------- stage B: own-sample weight assembly
            D = [sb.tile([128, 64], F32, tag=f"D{ch}", name=f"D{ch}") for ch in range(4)]
            PK = [sb.tile([128, 32], F32, tag=f"PK{ch}", name=f"PK{ch}") for ch in range(4)]
            PB = [sb.tile([128, 32], F32, tag=f"PB{ch}", name=f"PB{ch}") for ch in range(4)]
            for ch in range(4):
                for half in range(2):
                    r = 2 * ch + half
                    base = r * BLK
                    nc.sync.dma_start(
                        out=_ap(D[ch], half * 64 * 64, [[64, 64], [9, 4], [1, 9]]),
                        in_=_ap(agout[:], base, [[36, 64], [9, 4], [1, 9]]),
                    )
                    nc.sync.dma_start(
                        out=_ap(PK[ch], half * 64 * 32, [[32, 64], [1, 4]]),
                        in_=_ap(agout[:], base + BPK, [[4, 64], [1, 4]]),
                    )
                    nc.sync.dma_start(
                        out=_ap(PB[ch], half * 64 * 32, [[32, 64], [1, 1]]),
                        in_=_ap(agout[:], base + BPB, [[1, 64], [1, 1]]),
                    )

            permsb = sb.tile([128, 512], F32, tag="permsb", name="permsb")
            nc.sync.dma_start(
                out=_ap(permsb, 0, [[512, 128], [128, 4], [1, 128]]),
                in_=_ap(perm[:, :, :], 0, [[128, 128], [128 * 128, 4], [1, 128]]),
            )
            identsb = sb.tile([128, 128], F32, tag="identsb", name="identsb")
            nc.sync.dma_start(out=identsb[:, :], in_=ident[:, :])
            selsb = sb.tile([36, 9 * 128], F32, tag="selsb", name="selsb")
            nc.sync.dma_start(out=selsb[:, :], in_=selm[:, :])
            masksb = sb.tile([128, 128], F32, tag="masksb", name="masksb")
            nc.sync.dma_start(out=masksb[:, :], in_=maskm[:, :])
            S = [sb.tile([128, 9 * 128], BF16, tag=f"S{ch}", name=f"S{ch}") for ch in range(4)]
            wef = [sb.tile([128, 64], F32, tag=f"wef{ch}", name=f"wef{ch}") for ch in range(4)]
            wefT = [sb.tile([36, 128], F32, tag=f"wefT{ch}", name=f"wefT{ch}") for ch in range(4)]
            for ch in range(4):
                dp = ps.tile([128, 144], F32, tag="sA", name="sA")
                for m2 in range(4):
                    nc.tensor.matmul(
                        dp[:, m2 * 36:(m2 + 1) * 36],
                        permsb[:, m2 * 128:(m2 + 1) * 128],
                        D[ch][:, 0:36],
                        start=True,
                        stop=True,
                    )
                tmp = sb.tile([128, 36], F32, tag="weftmp", name="weftmp")
                nc.vector.tensor_scalar_mul(
                    wef[ch][:, 0:36], dp[:, 0:36], PK[ch][:, 0:1]
                )
                for m2 in range(1, 4):
                    nc.vector.tensor_scalar_mul(
                        tmp[:, :], dp[:, m2 * 36:(m2 + 1) * 36], PK[ch][:, m2:m2 + 1]
                    )
                    nc.vector.tensor_add(wef[ch][:, 0:36], wef[ch][:, 0:36], tmp[:, :])
                # expand W_eff -> block-diag S via PE select-matmuls + mask
                tp = ps.tile([36, 128], F32, tag="sA", name="sA")
                nc.tensor.matmul(
                    tp[:, :], wef[ch][:, 0:36], identsb[:, :], is_transpose=True,
                    start=True, stop=True,
                )
                nc.vector.tensor_copy(wefT[ch][:, :], tp[:, :])
                for t in range(9):
                    sps = ps.tile([128, 128], F32, tag="sB", name="sB")
                    nc.tensor.matmul(
                        sps[:, :],
                        selsb[:, t * 128:(t + 1) * 128],
                        wefT[ch][:, :],
                        start=True, stop=True,
                    )
                    nc.vector.tensor_tensor(
                        S[ch][:, t * 128:(t + 1) * 128], sps[:, :], masksb[:, :],
                        op=mybir.AluOpType.mult,
                    )

            # ---------------- stage C: reflect-pad edges + conv
            for ch in range(4):
                nc.vector.tensor_copy(
                    _ap(xp[ch], 1, [[4384, 128], [1, HW]]),
                    _ap(xp[ch], 2 * HWP + 1, [[4384, 128], [1, HW]]),
                )
                nc.vector.tensor_copy(
                    _ap(xp[ch], 65 * HWP + 1, [[4384, 128], [1, HW]]),
                    _ap(xp[ch], 63 * HWP + 1, [[4384, 128], [1, HW]]),
                )
                nc.vector.tensor_copy(
                    _ap(xp[ch], 0, [[4384, 128], [HWP, HWP]]),
                    _ap(xp[ch], 2, [[4384, 128], [HWP, HWP]]),
                )
                nc.vector.tensor_copy(
                    _ap(xp[ch], 65, [[4384, 128], [HWP, HWP]]),
                    _ap(xp[ch], 63, [[4384, 128], [HWP, HWP]]),
                )

            xb = [sbx.tile([128, 4384], BF16, tag=f"xb{ch}", name=f"xb{ch}")
                  for ch in range(4)]
            for ch in range(4):
                eng = nc.vector if ch % 2 == 0 else nc.scalar
                if ch % 2 == 0:
                    nc.vector.tensor_copy(xb[ch][:, :], xp[ch][:, :])
                else:
                    nc.scalar.activation(
                        xb[ch][:, :], xp[ch][:, :],
                        mybir.ActivationFunctionType.Copy,
                    )
            for ch in range(4):
                osb = sbo.tile([128, HW * HW], F32, tag="osb", name="osb")
                for sub in range(8):
                    cps = psc.tile([128, 512], F32, tag="cps", name="cps")
                    r0 = sub * 8
                    for tap in range(9):
                        di, dj = tap // 3, tap % 3
                        lhs = S[ch][:, tap * 128:(tap + 1) * 128]
                        rhs = _ap(xb[ch], (r0 + di) * HWP + dj,
                                  [[4384, 128], [HWP, 8], [1, HW]])
                        nc.tensor.matmul(
                            cps[:, :],
                            lhs,
                            rhs,
                            start=(tap == 0),
                            stop=(tap == 8),
                        )
                    nc.scalar.activation(
                        osb[:, r0 * HW:(r0 + 8) * HW], cps[:, :],
                        mybir.ActivationFunctionType.Identity,
                        bias=PB[ch][:, 0:1],
                    )
                nc.sync.dma_start(
                    out=out[ch * 128:(ch + 1) * 128, :], in_=osb[:, :]
                )

    _split_excess_waits(nc)
    return nc


_NC_CACHE = {}


def _get_nc():
    if "nc" not in _NC_CACHE:
        _NC_CACHE["nc"] = build_nc()
    return _NC_CACHE["nc"]


def make_in_maps(inputs):
    """Host-side shard/layout prep (pure layout: transpose/reshape/slice)."""
    style = np.asarray(inputs["style_encoding"], np.float32)
    pred = np.asarray(inputs["predicted"], np.float32)
    w1 = np.asarray(inputs["dw1_w"], np.float32).reshape(512, 512)
    w2 = np.asarray(inputs["dw2_w"], np.float32).reshape(2048, 512, 2, 2)
    pk1 = np.asarray(inputs["pk1_w"], np.float32).reshape(512, 512)
    pk2 = np.asarray(inputs["pk2_w"], np.float32).reshape(2048, 512)
    pb1 = np.asarray(inputs["pb1_w"], np.float32).reshape(512, 512)
    pb2 = np.asarray(inputs["pb2_w"], np.float32).reshape(512, 512)

    w1t = np.ascontiguousarray(w1.T)
    pk1t = np.ascontiguousarray(pk1.T)
    pb1t = np.ascontiguousarray(pb1.T)
    w2t_full = (
        w2.reshape(2048, 4, 128, 2, 2)
        .transpose(1, 3, 4, 2, 0)          # [it, di, dj, 128, o]
        .reshape(16, 128, 2048)
    )
    pk2t_full = np.ascontiguousarray(pk2.T).reshape(4, 128, 2048)
    pb2t_full = np.ascontiguousarray(pb2.T).reshape(4, 128, 512)
    st_all = np.ascontiguousarray(
        style.transpose(1, 0, 2, 3).reshape(512, N * NPOS)
    )

    permm = np.zeros((4, 128, 128), np.float32)
    for m2 in range(4):
        for p in range(128):
            permm[m2, 4 * (p // 4) + m2, p] = 1.0
    identm = np.eye(128, dtype=np.float32)
    selm = np.zeros((36, 9, 128), np.float32)
    for t in range(9):
        for p in range(128):
            selm[(p % 4) * 9 + t, t, p] = 1.0
    selm = selm.reshape(36, 9 * 128)
    maskm = np.zeros((128, 128), np.float32)
    for p in range(128):
        for col in range(128):
            if p // 4 == col // 4:
                maskm[p, col] = 1.0

    in_maps = []
    for c in range(N):
        m = {
            "style_all": st_all,
            "xin": np.ascontiguousarray(pred[c].reshape(512, HW * HW)),
            "w1t": w1t,
            "pk1t": pk1t,
            "pb1t": pb1t,
            "w2t": np.ascontiguousarray(w2t_full[:, :, c * OSL:(c + 1) * OSL]),
            "pk2t": np.ascontiguousarray(pk2t_full[:, :, c * PKSL:(c + 1) * PKSL]),
            "pb2t": np.ascontiguousarray(pb2t_full[:, :, c * PBSL:(c + 1) * PBSL]),
            "b1": np.asarray(inputs["dw1_b"], np.float32),
            "bk1": np.asarray(inputs["pk1_b"], np.float32),
            "bb1": np.asarray(inputs["pb1_b"], np.float32),
            "b2s": np.asarray(inputs["dw2_b"], np.float32)[c * OSL:(c + 1) * OSL],
            "bk2s": np.asarray(inputs["pk2_b"], np.float32)[c * PKSL:(c + 1) * PKSL],
            "bb2s": np.asarray(inputs["pb2_b"], np.float32)[c * PBSL:(c + 1) * PBSL],
            "perm": permm,
            "ident": identm,
            "selm": selm,
            "maskm": maskm,
        }
        in_maps.append(m)
    return in_maps


def kernel(**inputs):
    install_profile_shim()
    from concourse.bass_utils import run_bass_kernel_spmd

    nc = _get_nc()
    in_maps = make_in_maps(inputs)
    res = run_bass_kernel_spmd(nc, in_maps, core_ids=list(range(N)))
    outs = [np.asarray(res.results[c]["out"]).reshape(COUT, HW, HW)
            for c in range(N)]
    return np.stack(outs, axis=0).astype(np.float32)



# revision 6
# speedup vs baseline: 1.4465x; 1.4465x over previous
"""AdaConv Trainium2 kernel — 8-core SPMD, data-parallel over batch.

Per core c (sample c):
  Stage A: kernel-prediction net for ALL 8 samples, layer-2 weights sharded
           by output channel across cores; AllToAll redistributes so each
           core ends with the full dynamic weights for its own sample.
  Stage B: build fused per-sample conv weights W_eff = PK o D and expand
           them into block-diagonal stationary matrices S via PE select
           matmuls + mask.
  Stage C: main grouped conv (128 groups of 4->4 ch, 3x3, reflect pad) as
           9 PSUM-accumulated bf16 matmuls per 128-channel chunk.

Perf structure (v2):
  - All kernel-prediction weights are bf16 (host-converted) and loaded
    FIRST on the SP HWDGE ring, so the AllToAll fires within ~10us.
  - The big `predicted` input is loaded contiguously (128 descriptors of
    16KB per chunk) on the ACT HWDGE ring so it never blocks the small
    latency-critical SP-ring transfers. Pad+f32->bf16 convert happens
    on-chip (vector/scalar), overlapped with the collective.
  - Stage C conv runs chunk-pipelined right after the collective.
"""
import sys
import types

sys.path.insert(0, "/opt/trn_rl_repo")

import numpy as np
import ml_dtypes

import concourse.bass as bass
import concourse.mybir as mybir

N = 8          # batch == cores
CIN = 512
COUT = 512
HW = 64        # spatial
HWP = 66       # padded
NPOS = 16      # style spatial 4x4
OSL = 2048 // N      # dw2 out-channel slice per core (256)
PKSL = 2048 // N     # pk2 slice (256)
PBSL = 512 // N      # pb2 slice (64)
# AllToAll per-rank block: [dw 256*9 | pk 256 | pb 64]
BDW = 0
BPK = OSL * 9              # 2304
BPB = BPK + PKSL           # 2560
BLK = BPB + PBSL           # 2624
AG_SZ = N * BLK            # 20992

F32 = mybir.dt.float32
BF16 = mybir.dt.bfloat16
BF16_NP = ml_dtypes.bfloat16


# ---------------------------------------------------------------- tile patch
def _install_tile_patch():
    """walrus here rejects Drain instructions with >1 sync-wait; spread the
    Tile tail-drain waits over individual SP nops."""
    import concourse.tile as tile_mod
    from concourse.vector_clock import ScopedClock

    def _patched(self, tick_clock, wait_clock):
        nc = self.nc
        drain_inst = nc.sync.drain()
        wait_clock.add_sem_waits(
            drain_inst.ins, ScopedClock({None: tick_clock.global_clock})
        )
        waits = list(drain_inst.ins.sync_info.on_wait or [])
        if len(waits) > 1:
            drain_inst.ins.sync_info.on_wait = waits[:1]
            for w in waits[1:]:
                nop = nc.sync.nop(nofuse=True, hint="tail_wait_split")
                if nop.ins.sync_info is None:
                    nop.ins.sync_info = mybir.SyncInfo(on_wait=[w], on_update=[])
                else:
                    nop.ins.sync_info.on_wait = [w]
        nc.all_engine_barrier()
        assert self.sems is not None
        popped = nc._tile_sem_poison_stack.pop()
        assert popped is self._sem_poison
        nc.clear_and_free_semaphores(list(self.sems.allocated().values()))
        nc.all_engine_barrier()

    tile_mod.TileContext._drain_and_barrier = _patched


_install_tile_patch()
from concourse.tile import TileContext  # noqa: E402


def install_profile_shim():
    """antenv.axon_hooks is missing from this image; recreate it so
    run_bass_kernel_spmd(trace=True) can capture NTFF profiles."""
    if "antenv.axon_hooks" in sys.modules:
        return
    import antenv

    mod = types.ModuleType("antenv.axon_hooks")
    mod._hook = None
    mod.set_axon_ntff_profile_hook = lambda h: setattr(mod, "_hook", h)
    mod.get_axon_ntff_profile_hook = lambda: mod._hook
    sys.modules["antenv.axon_hooks"] = mod
    antenv.axon_hooks = mod
    try:
        if "/root/.axon_site" not in sys.path:
            sys.path.insert(0, "/root/.axon_site")
        from trn_agent_boot.trn_boot import _ntff_profile_via_ctypes

        hook = _ntff_profile_via_ctypes("/opt/axon/libaxon_pjrt.so")
        mod.set_axon_ntff_profile_hook(hook)
    except Exception:
        pass


def _ap(t_ap, offset, dims):
    """Custom flat AP over a tile's underlying tensor."""
    return bass.AP(t_ap.tensor, offset, [list(d) for d in dims])


def _pt(t):
    """Physical partition pitch (elements) of a tile."""
    return t[:, :].ap[0][0]


def _split_excess_waits(nc, max_waits=1):
    """This walrus build rejects instructions carrying more than ~1 sync-wait.
    Move excess waits onto same-engine NoOps inserted just before."""
    n_split = 0
    for f in nc.m.functions:
        for bb in f.blocks:
            newlist = []
            for inst in bb.instructions:
                si = getattr(inst, "sync_info", None)
                if si is not None and si.on_wait and len(si.on_wait) > max_waits:
                    waits = list(si.on_wait)
                    for k, w in enumerate(waits[max_waits:]):
                        nop = mybir.InstNoOp(
                            name=f"{inst.name}_ws{k}",
                            engine=inst.engine,
                            bass_nofuse=True,
                            sync_info=mybir.SyncInfo(on_wait=[w], on_update=[]),
                        )
                        newlist.append(nop)
                        n_split += 1
                    si.on_wait = waits[:max_waits]
                newlist.append(inst)
            try:
                bb.instructions[:] = newlist
            except TypeError:
                bb.set_instructions(newlist)
    return n_split


def build_nc():
    nc = bass.Bass(target_bir_lowering=False)

    style = nc.declare_dram_parameter("style_all", [CIN, N * NPOS], BF16, isOutput=False)
    xin = nc.declare_dram_parameter("xin", [CIN, HW * HW], F32, isOutput=False)
    w1t = nc.declare_dram_parameter("w1t", [CIN, CIN], BF16, isOutput=False)
    pk1t = nc.declare_dram_parameter("pk1t", [CIN, CIN], BF16, isOutput=False)
    pb1t = nc.declare_dram_parameter("pb1t", [CIN, CIN], BF16, isOutput=False)
    w2t = nc.declare_dram_parameter("w2t", [16, 128, OSL], BF16, isOutput=False)
    pk2t = nc.declare_dram_parameter("pk2t", [4, 128, PKSL], BF16, isOutput=False)
    pb2t = nc.declare_dram_parameter("pb2t", [4, 128, PBSL], BF16, isOutput=False)
    b1 = nc.declare_dram_parameter("b1", [CIN], F32, isOutput=False)
    bk1 = nc.declare_dram_parameter("bk1", [CIN], F32, isOutput=False)
    bb1 = nc.declare_dram_parameter("bb1", [CIN], F32, isOutput=False)
    b2s = nc.declare_dram_parameter("b2s", [OSL], F32, isOutput=False)
    bk2s = nc.declare_dram_parameter("bk2s", [PKSL], F32, isOutput=False)
    bb2s = nc.declare_dram_parameter("bb2s", [PBSL], F32, isOutput=False)
    perm = nc.declare_dram_parameter("perm", [4, 128, 128], F32, isOutput=False)
    ident = nc.declare_dram_parameter("ident", [128, 128], F32, isOutput=False)
    selm = nc.declare_dram_parameter("selm", [36, 9 * 128], F32, isOutput=False)
    maskm = nc.declare_dram_parameter("maskm", [128, 128], F32, isOutput=False)
    out = nc.declare_dram_parameter("out", [COUT, HW * HW], F32, isOutput=True)

    with TileContext(nc) as tc:
        with (
            tc.tile_pool(name="sb", bufs=1) as sb,
            tc.tile_pool(name="sbx", bufs=1) as sbx,
            tc.tile_pool(name="sbo", bufs=2) as sbo,
            tc.tile_pool(name="ps", bufs=2, space="PSUM") as ps,
            tc.tile_pool(name="psc", bufs=4, space="PSUM") as psc,
            tc.tile_pool(name="dram", bufs=1, space="DRAM") as dram,
        ):
            # ================ SP-ring loads: small latency-critical weights
            st = []
            for i in range(4):
                t = sb.tile([128, N * NPOS], BF16, tag=f"st{i}", name=f"st{i}")
                nc.sync.dma_start(out=t[:, :], in_=style[i * 128:(i + 1) * 128, :])
                st.append(t)
            w1sb = sb.tile([128, 4 * CIN], BF16, tag="w1sb", name="w1sb")
            for it in range(4):
                nc.sync.dma_start(
                    out=_ap(w1sb, it * CIN, [[_pt(w1sb), 128], [1, CIN]]),
                    in_=w1t[it * 128:(it + 1) * 128, :],
                )
            b1t = sb.tile([128, 4], F32, tag="b1t", name="b1t")
            for ot in range(4):
                nc.sync.dma_start(
                    out=b1t[:, ot:ot + 1], in_=_ap(b1[:], ot * 128, [[1, 128], [1, 1]])
                )
            w2sb = sb.tile([128, 16 * OSL], BF16, tag="w2sb", name="w2sb")
            nc.sync.dma_start(
                out=_ap(w2sb, 0, [[_pt(w2sb), 128], [OSL, 16], [1, OSL]]),
                in_=_ap(w2t[:, :, :], 0, [[OSL, 128], [128 * OSL, 16], [1, OSL]]),
            )
            b2t = sb.tile([128, 2], F32, tag="b2t", name="b2t")
            for o2 in range(2):
                nc.sync.dma_start(
                    out=b2t[:, o2:o2 + 1],
                    in_=_ap(b2s[:], o2 * 128, [[1, 128], [1, 1]]),
                )

            pw = {}
            for tagp, wt_param, bias_param in (
                ("pk1", pk1t, bk1), ("pb1", pb1t, bb1)
            ):
                wsb = sb.tile([128, 4 * CIN], BF16, tag=f"{tagp}w", name=f"{tagp}w")
                for it in range(4):
                    nc.sync.dma_start(
                        out=_ap(wsb, it * CIN, [[_pt(wsb), 128], [1, CIN]]),
                        in_=wt_param[it * 128:(it + 1) * 128, :],
                    )
                bt = sb.tile([128, 4], F32, tag=f"{tagp}b", name=f"{tagp}b")
                for ot in range(4):
                    nc.sync.dma_start(
                        out=bt[:, ot:ot + 1],
                        in_=_ap(bias_param[:], ot * 128, [[1, 128], [1, 1]]),
                    )
                pw[tagp] = (wsb, bt)

            pk2sb = sb.tile([128, 4 * PKSL], BF16, tag="pk2sb", name="pk2sb")
            nc.sync.dma_start(
                out=_ap(pk2sb, 0, [[_pt(pk2sb), 128], [PKSL, 4], [1, PKSL]]),
                in_=_ap(pk2t[:, :, :], 0, [[PKSL, 128], [128 * PKSL, 4], [1, PKSL]]),
            )
            bk2t = sb.tile([128, 2], F32, tag="bk2t", name="bk2t")
            for o2 in range(2):
                nc.sync.dma_start(
                    out=bk2t[:, o2:o2 + 1],
                    in_=_ap(bk2s[:], o2 * 128, [[1, 128], [1, 1]]),
                )
            pb2sb = sb.tile([128, 4 * PBSL], BF16, tag="pb2sb", name="pb2sb")
            nc.sync.dma_start(
                out=_ap(pb2sb, 0, [[_pt(pb2sb), 128], [PBSL, 4], [1, PBSL]]),
                in_=_ap(pb2t[:, :, :], 0, [[PBSL, 128], [128 * PBSL, 4], [1, PBSL]]),
            )
            bb2t = sb.tile([64, 1], F32, tag="bb2t", name="bb2t")
            nc.sync.dma_start(
                out=bb2t[:, 0:1], in_=_ap(bb2s[:], 0, [[1, 64], [1, 1]])
            )
            permsb = sb.tile([128, 512], F32, tag="permsb", name="permsb")
            nc.sync.dma_start(
                out=_ap(permsb, 0, [[_pt(permsb), 128], [128, 4], [1, 128]]),
                in_=_ap(perm[:, :, :], 0, [[128, 128], [128 * 128, 4], [1, 128]]),
            )
            identsb = sb.tile([128, 128], F32, tag="identsb", name="identsb")
            nc.sync.dma_start(out=identsb[:, :], in_=ident[:, :])
            selsb = sb.tile([36, 9 * 128], F32, tag="selsb", name="selsb")
            nc.sync.dma_start(out=selsb[:, :], in_=selm[:, :])
            masksb = sb.tile([128, 128], F32, tag="masksb", name="masksb")
            nc.sync.dma_start(out=masksb[:, :], in_=maskm[:, :])

            # ================ ACT-ring loads: big contiguous input chunks
            xsb = [sbx.tile([128, HW * HW], F32, tag=f"xsb{ch}", name=f"xsb{ch}")
                   for ch in range(4)]
            for ch in range(4):
                nc.scalar.dma_start(
                    out=xsb[ch][:, :], in_=xin[ch * 128:(ch + 1) * 128, :]
                )

            # ================ stage A: layer-1 (h = lrelu(W1 s + b1))
            h = [sb.tile([128, N * NPOS], BF16, tag=f"h{ot}", name=f"h{ot}") for ot in range(4)]
            for ot in range(4):
                hp = ps.tile([128, N * NPOS], F32, tag="sA", name="sA")
                for it in range(4):
                    nc.tensor.matmul(
                        hp[:, :],
                        _ap(w1sb, it * CIN + ot * 128, [[_pt(w1sb), 128], [1, 128]]),
                        st[it][:, :],
                        start=(it == 0),
                        stop=(it == 3),
                    )
                nc.scalar.activation(
                    h[ot][:, :], hp[:, :], mybir.ActivationFunctionType.Identity,
                    bias=b1t[:, ot:ot + 1],
                )
                zt = sb.tile([128, N * NPOS], BF16, tag="zt", name="zt")
                nc.vector.tensor_scalar_mul(zt[:, :], h[ot][:, :], 0.01)
                nc.vector.tensor_max(h[ot][:, :], h[ot][:, :], zt[:, :])

            # ---------------- stage A: dw2 slice for all samples
            agin = dram.tile([AG_SZ], F32)
            agout = dram.tile([AG_SZ], F32)
            dwc = [sb.tile([128, 96], F32, tag=f"dwc{o2}", name=f"dwc{o2}") for o2 in range(2)]
            for o2 in range(2):
                dps = ps.tile([128, N * 9], F32, tag="sA", name="sA")
                for kt in range(16):
                    it, tap = kt // 4, kt % 4
                    di, dj = tap // 2, tap % 2
                    rhs = _ap(h[it], di * 4 + dj,
                              [[_pt(h[it]), 128], [NPOS, N], [4, 3], [1, 3]])
                    nc.tensor.matmul(
                        dps[:, :],
                        _ap(w2sb, kt * OSL + o2 * 128, [[_pt(w2sb), 128], [1, 128]]),
                        rhs,
                        start=(kt == 0),
                        stop=(kt == 15),
                    )
                nc.scalar.activation(
                    dwc[o2][:, 0:72], dps[:, :], mybir.ActivationFunctionType.Identity,
                    bias=b2t[:, o2:o2 + 1],
                )
                # agin[n*BLK + (o2*128+p)*9 + pos] = dwc[o2][p, n*9+pos]
                nc.sync.dma_start(
                    out=_ap(agin[:], o2 * 128 * 9, [[9, 128], [BLK, N], [1, 9]]),
                    in_=_ap(dwc[o2], 0, [[_pt(dwc[o2]), 128], [9, N], [1, 9]]),
                )

            # ---------------- stage A: pooled-style path (pk / pb)
            sp = [sb.tile([128, N], BF16, tag=f"sp{i}", name=f"sp{i}") for i in range(4)]
            spf = sb.tile([128, N], F32, tag="spf", name="spf")
            for i in range(4):
                nc.vector.tensor_reduce(
                    spf[:, :],
                    _ap(st[i], 0, [[_pt(st[i]), 128], [NPOS, N], [1, NPOS]]),
                    axis=mybir.AxisListType.X,
                    op=mybir.AluOpType.add,
                )
                nc.vector.tensor_scalar_mul(sp[i][:, :], spf[:, :], 1.0 / NPOS)

            def layer1(tagp):
                wsb, bt = pw[tagp]
                acts = []
                for ot in range(4):
                    ap_ = ps.tile([128, N], F32, tag="sA", name="sA")
                    for it in range(4):
                        nc.tensor.matmul(
                            ap_[:, :],
                            _ap(wsb, it * CIN + ot * 128, [[_pt(wsb), 128], [1, 128]]),
                            sp[it][:, :],
                            start=(it == 0),
                            stop=(it == 3),
                        )
                    a = sb.tile([128, N], BF16, tag=f"{tagp}a{ot}", name=f"{tagp}a{ot}")
                    nc.scalar.activation(
                        a[:, :], ap_[:, :], mybir.ActivationFunctionType.Identity,
                        bias=bt[:, ot:ot + 1],
                    )
                    zt2 = sb.tile([128, N], BF16, tag="zt2", name="zt2")
                    nc.vector.tensor_scalar_mul(zt2[:, :], a[:, :], 0.01)
                    nc.vector.tensor_max(a[:, :], a[:, :], zt2[:, :])
                    acts.append(a)
                return acts

            a1 = layer1("pk1")
            c1 = layer1("pb1")

            for o2 in range(2):
                pp = ps.tile([128, N], F32, tag="sA", name="sA")
                for it in range(4):
                    nc.tensor.matmul(
                        pp[:, :],
                        _ap(pk2sb, it * PKSL + o2 * 128, [[_pt(pk2sb), 128], [1, 128]]),
                        a1[it][:, :],
                        start=(it == 0),
                        stop=(it == 3),
                    )
                pkc = sb.tile([128, 32], F32, tag=f"pkc{o2}", name=f"pkc{o2}")
                nc.scalar.activation(
                    pkc[:, 0:8], pp[:, :], mybir.ActivationFunctionType.Identity,
                    bias=bk2t[:, o2:o2 + 1],
                )
                nc.sync.dma_start(
                    out=_ap(agin[:], BPK + o2 * 128, [[1, 128], [BLK, N]]),
                    in_=_ap(pkc, 0, [[_pt(pkc), 128], [1, N]]),
                )

            pbp = ps.tile([64, N], F32, tag="sA", name="sA")
            for it in range(4):
                nc.tensor.matmul(
                    pbp[:, :],
                    _ap(pb2sb, it * PBSL, [[_pt(pb2sb), 128], [1, PBSL]]),
                    c1[it][:, :],
                    start=(it == 0),
                    stop=(it == 3),
                )
            pbc = sb.tile([64, 32], F32, tag="pbc", name="pbc")
            nc.scalar.activation(
                pbc[:, 0:8], pbp[:, :], mybir.ActivationFunctionType.Identity,
                bias=bb2t[:, 0:1],
            )
            nc.sync.dma_start(
                out=_ap(agin[:], BPB, [[1, 64], [BLK, N]]),
                in_=_ap(pbc, 0, [[_pt(pbc), 64], [1, N]]),
            )

            # ================ pad + f32->bf16 convert (overlaps collective)
            xb = [sbx.tile([128, 4384], BF16, tag=f"xb{ch}", name=f"xb{ch}")
                  for ch in range(4)]
            # All pad work runs on the scalar (ACT) queue: it has nothing
            # else to do after stage A, so a late xsb chunk never blocks
            # stage-B/C vector work or the conv PSUM drain.
            for ch in range(4):
                ptx = _pt(xb[ch])
                interior_out = _ap(xb[ch], HWP + 1, [[ptx, 128], [HWP, HW], [1, HW]])
                interior_in = _ap(xsb[ch], 0, [[_pt(xsb[ch]), 128], [HW, HW], [1, HW]])
                nc.scalar.activation(
                    interior_out, interior_in,
                    mybir.ActivationFunctionType.Copy,
                )
                # reflect rows (row0 <- row2, row65 <- row63)
                nc.scalar.activation(
                    _ap(xb[ch], 1, [[ptx, 128], [1, HW]]),
                    _ap(xb[ch], 2 * HWP + 1, [[ptx, 128], [1, HW]]),
                    mybir.ActivationFunctionType.Copy,
                )
                nc.scalar.activation(
                    _ap(xb[ch], 65 * HWP + 1, [[ptx, 128], [1, HW]]),
                    _ap(xb[ch], 63 * HWP + 1, [[ptx, 128], [1, HW]]),
                    mybir.ActivationFunctionType.Copy,
                )
                # reflect cols (col0 <- col2, col65 <- col63)
                nc.scalar.activation(
                    _ap(xb[ch], 0, [[ptx, 128], [HWP, HWP]]),
                    _ap(xb[ch], 2, [[ptx, 128], [HWP, HWP]]),
                    mybir.ActivationFunctionType.Copy,
                )
                nc.scalar.activation(
                    _ap(xb[ch], 65, [[ptx, 128], [HWP, HWP]]),
                    _ap(xb[ch], 63, [[ptx, 128], [HWP, HWP]]),
                    mybir.ActivationFunctionType.Copy,
                )

            # ================ AllToAll: core c receives, from every rank r,
            # rank r's o-slice of sample c's dynamic weights.
            nc.gpsimd.collective_compute(
                "AllToAll",
                mybir.AluOpType.bypass,
                replica_groups=[list(range(N))],
                ins=[agin[:].opt()],
                outs=[agout[:].opt()],
            )

            # ================ stage B + stage C, chunk-pipelined
            D = [sb.tile([128, 64], F32, tag=f"D{ch}", name=f"D{ch}") for ch in range(4)]
            PK = [sb.tile([128, 32], F32, tag=f"PK{ch}", name=f"PK{ch}") for ch in range(4)]
            PB = [sb.tile([128, 32], F32, tag=f"PB{ch}", name=f"PB{ch}") for ch in range(4)]
            for ch in range(4):
                for half in range(2):
                    r = 2 * ch + half
                    base = r * BLK
                    nc.sync.dma_start(
                        out=_ap(D[ch], half * 64 * _pt(D[ch]), [[_pt(D[ch]), 64], [9, 4], [1, 9]]),
                        in_=_ap(agout[:], base, [[36, 64], [9, 4], [1, 9]]),
                    )
                    nc.sync.dma_start(
                        out=_ap(PK[ch], half * 64 * _pt(PK[ch]), [[_pt(PK[ch]), 64], [1, 4]]),
                        in_=_ap(agout[:], base + BPK, [[4, 64], [1, 4]]),
                    )
                    nc.sync.dma_start(
                        out=_ap(PB[ch], half * 64 * _pt(PB[ch]), [[_pt(PB[ch]), 64], [1, 1]]),
                        in_=_ap(agout[:], base + BPB, [[1, 64], [1, 1]]),
                    )

            S = [sb.tile([128, 9 * 128], BF16, tag=f"S{ch}", name=f"S{ch}") for ch in range(4)]
            wef = [sb.tile([128, 64], F32, tag=f"wef{ch}", name=f"wef{ch}") for ch in range(4)]
            wefT = [sb.tile([36, 128], F32, tag=f"wefT{ch}", name=f"wefT{ch}") for ch in range(4)]

            def build_S(ch):
                dp = ps.tile([128, 144], F32, tag="sA", name="sA")
                for m2 in range(4):
                    nc.tensor.matmul(
                        dp[:, m2 * 36:(m2 + 1) * 36],
                        permsb[:, m2 * 128:(m2 + 1) * 128],
                        D[ch][:, 0:36],
                        start=True,
                        stop=True,
                    )
                tmp = sb.tile([128, 36], F32, tag="weftmp", name="weftmp")
                nc.vector.tensor_scalar_mul(
                    wef[ch][:, 0:36], dp[:, 0:36], PK[ch][:, 0:1]
                )
                for m2 in range(1, 4):
                    nc.vector.tensor_scalar_mul(
                        tmp[:, :], dp[:, m2 * 36:(m2 + 1) * 36], PK[ch][:, m2:m2 + 1]
                    )
                    nc.vector.tensor_add(wef[ch][:, 0:36], wef[ch][:, 0:36], tmp[:, :])
                # expand W_eff -> block-diag S via PE select-matmuls + mask
                tp = ps.tile([36, 128], F32, tag="sA", name="sA")
                nc.tensor.matmul(
                    tp[:, :], wef[ch][:, 0:36], identsb[:, :], is_transpose=True,
                    start=True, stop=True,
                )
                nc.vector.tensor_copy(wefT[ch][:, :], tp[:, :])
                for t in range(9):
                    sps = ps.tile([128, 128], F32, tag="sB", name="sB")
                    nc.tensor.matmul(
                        sps[:, :],
                        selsb[:, t * 128:(t + 1) * 128],
                        wefT[ch][:, :],
                        start=True, stop=True,
                    )
                    nc.vector.tensor_tensor(
                        S[ch][:, t * 128:(t + 1) * 128], sps[:, :], masksb[:, :],
                        op=mybir.AluOpType.mult,
                    )

            for ch in range(4):
                build_S(ch)
                ptx = _pt(xb[ch])
                osb = sbo.tile([128, HW * HW], F32, tag="osb", name="osb")
                for sub in range(8):
                    cps = psc.tile([128, 512], F32, tag="cps", name="cps")
                    r0 = sub * 8
                    for tap in range(9):
                        di, dj = tap // 3, tap % 3
                        lhs = S[ch][:, tap * 128:(tap + 1) * 128]
                        rhs = _ap(xb[ch], (r0 + di) * HWP + dj,
                                  [[ptx, 128], [HWP, 8], [1, HW]])
                        nc.tensor.matmul(
                            cps[:, :],
                            lhs,
                            rhs,
                            start=(tap == 0),
                            stop=(tap == 8),
                        )
                    nc.vector.tensor_scalar_add(
                        osb[:, r0 * HW:(r0 + 8) * HW], cps[:, :],
                        PB[ch][:, 0:1],
                    )
                nc.scalar.dma_start(
                    out=out[ch * 128:(ch + 1) * 128, :], in_=osb[:, :]
                )

    _split_excess_waits(nc)
    return nc


_NC_CACHE = {}


def _get_nc():
    if "nc" not in _NC_CACHE:
        _NC_CACHE["nc"] = build_nc()
    return _NC_CACHE["nc"]


def make_in_maps(inputs):
    """Host-side shard/layout prep (pure layout: transpose/reshape/slice)."""
    style = np.asarray(inputs["style_encoding"], np.float32)
    pred = np.asarray(inputs["predicted"], np.float32)
    w1 = np.asarray(inputs["dw1_w"], np.float32).reshape(512, 512)
    w2 = np.asarray(inputs["dw2_w"], np.float32).reshape(2048, 512, 2, 2)
    pk1 = np.asarray(inputs["pk1_w"], np.float32).reshape(512, 512)
    pk2 = np.asarray(inputs["pk2_w"], np.float32).reshape(2048, 512)
    pb1 = np.asarray(inputs["pb1_w"], np.float32).reshape(512, 512)
    pb2 = np.asarray(inputs["pb2_w"], np.float32).reshape(512, 512)

    w1t = np.ascontiguousarray(w1.T).astype(BF16_NP)
    pk1t = np.ascontiguousarray(pk1.T).astype(BF16_NP)
    pb1t = np.ascontiguousarray(pb1.T).astype(BF16_NP)
    w2t_full = (
        w2.reshape(2048, 4, 128, 2, 2)
        .transpose(1, 3, 4, 2, 0)          # [it, di, dj, 128, o]
        .reshape(16, 128, 2048)
    ).astype(BF16_NP)
    pk2t_full = np.ascontiguousarray(pk2.T).reshape(4, 128, 2048).astype(BF16_NP)
    pb2t_full = np.ascontiguousarray(pb2.T).reshape(4, 128, 512).astype(BF16_NP)
    st_all = np.ascontiguousarray(
        style.transpose(1, 0, 2, 3).reshape(512, N * NPOS)
    ).astype(BF16_NP)

    permm = np.zeros((4, 128, 128), np.float32)
    for m2 in range(4):
        for p in range(128):
            permm[m2, 4 * (p // 4) + m2, p] = 1.0
    identm = np.eye(128, dtype=np.float32)
    selm = np.zeros((36, 9, 128), np.float32)
    for t in range(9):
        for p in range(128):
            selm[(p % 4) * 9 + t, t, p] = 1.0
    selm = selm.reshape(36, 9 * 128)
    maskm = np.zeros((128, 128), np.float32)
    for p in range(128):
        for col in range(128):
            if p // 4 == col // 4:
                maskm[p, col] = 1.0

    in_maps = []
    for c in range(N):
        m = {
            "style_all": st_all,
            "xin": np.ascontiguousarray(pred[c].reshape(512, HW * HW)),
            "w1t": w1t,
            "pk1t": pk1t,
            "pb1t": pb1t,
            "w2t": np.ascontiguousarray(w2t_full[:, :, c * OSL:(c + 1) * OSL]),
            "pk2t": np.ascontiguousarray(pk2t_full[:, :, c * PKSL:(c + 1) * PKSL]),
            "pb2t": np.ascontiguousarray(pb2t_full[:, :, c * PBSL:(c + 1) * PBSL]),
            "b1": np.asarray(inputs["dw1_b"], np.float32),
            "bk1": np.asarray(inputs["pk1_b"], np.float32),
            "bb1": np.asarray(inputs["pb1_b"], np.float32),
            "b2s": np.asarray(inputs["dw2_b"], np.float32)[c * OSL:(c + 1) * OSL],
            "bk2s": np.asarray(inputs["pk2_b"], np.float32)[c * PKSL:(c + 1) * PKSL],
            "bb2s": np.asarray(inputs["pb2_b"], np.float32)[c * PBSL:(c + 1) * PBSL],
            "perm": permm,
            "ident": identm,
            "selm": selm,
            "maskm": maskm,
        }
        in_maps.append(m)
    return in_maps


def kernel(**inputs):
    install_profile_shim()
    from concourse.bass_utils import run_bass_kernel_spmd

    nc = _get_nc()
    in_maps = make_in_maps(inputs)
    res = run_bass_kernel_spmd(nc, in_maps, core_ids=list(range(N)))
    outs = [np.asarray(res.results[c]["out"]).reshape(COUT, HW, HW)
            for c in range(N)]
    return np.stack(outs, axis=0).astype(np.float32)


# revision 8
# speedup vs baseline: 1.7635x; 1.2191x over previous
"""AdaConv Trainium2 kernel — 8-core SPMD, data-parallel over batch.

Per core c (sample c):
  Stage A: kernel-prediction net for ALL 8 samples, layer-2 weights sharded
           by output channel across cores; AllToAll redistributes so each
           core ends with the full dynamic weights for its own sample.
  Stage B: build fused per-sample conv weights W_eff = PK o D and expand
           them into block-diagonal stationary matrices S via PE select
           matmuls + mask.
  Stage C: main grouped conv (128 groups of 4->4 ch, 3x3, reflect pad) as
           9 PSUM-accumulated bf16 matmuls per 128-channel chunk.

Perf structure (v3):
  - All kernel-prediction weights host-packed into 3 bf16 tensors in final
    SBUF layout -> 3 contiguous DMAs (few hundred descriptors) instead of
    thousands of tiny strided ones. Biases/constants packed into one f32
    tensor.
  - The big `predicted` input loads contiguously on the ACT HWDGE ring;
    pad+f32->bf16 convert on-chip (scalar engine), overlapped with the
    collective.
  - AllToAll payload in bf16 with dw+pk interleaved per channel so the
    write is runs-of-10 and the read is one contiguous 40-element run per
    channel.
  - Stage B matmuls in bf16; output stored bf16 (host converts to f32).
"""
import sys
import types

sys.path.insert(0, "/opt/trn_rl_repo")

import numpy as np
import ml_dtypes

import concourse.bass as bass
import concourse.mybir as mybir

N = 8          # batch == cores
CIN = 512
COUT = 512
HW = 64        # spatial
HWP = 66       # padded
NPOS = 16      # style spatial 4x4
OSL = 2048 // N      # dw2 out-channel slice per core (256)
PKSL = 2048 // N     # pk2 slice (256)
PBSL = 512 // N      # pb2 slice (64)
# AllToAll per-rank block: [ch (256): 9 dw + 1 pk | pb 64]
BLK = OSL * 10 + PBSL      # 2624
BPB = OSL * 10             # 2560
AG_SZ = N * BLK            # 20992

# wbA columns (bf16): [st 512 | w1 2048]
OFF_ST = 0
OFF_W1 = 512
WBA_W = 2560
# wbB columns (bf16): [w2 4096]
WBB_W = 16 * OSL           # 4096
# wbC columns (bf16): [pk1 2048 | pb1 2048 | pk2 1024 | pb2 256 | perm 512]
OFF_PK1 = 0
OFF_PB1 = 2048
OFF_PK2 = 4096
OFF_PB2 = 5120
OFF_PERM = 5376
WBC_W = 5888
# cf columns (f32): [b1 4 | b2 2 | bk1 4 | bb1 4 | bk2 2 | bb2 1 | ident 128 | mask 128]
CF_B1 = 0
CF_B2 = 4
CF_BK1 = 6
CF_BB1 = 10
CF_BK2 = 14
CF_BB2 = 16
CF_IDENT = 17
CF_MASK = 145
CF_W = 273

F32 = mybir.dt.float32
BF16 = mybir.dt.bfloat16
BF16_NP = ml_dtypes.bfloat16


# ---------------------------------------------------------------- tile patch
def _install_tile_patch():
    """walrus here rejects Drain instructions with >1 sync-wait; spread the
    Tile tail-drain waits over individual SP nops."""
    import concourse.tile as tile_mod
    from concourse.vector_clock import ScopedClock

    def _patched(self, tick_clock, wait_clock):
        nc = self.nc
        drain_inst = nc.sync.drain()
        wait_clock.add_sem_waits(
            drain_inst.ins, ScopedClock({None: tick_clock.global_clock})
        )
        waits = list(drain_inst.ins.sync_info.on_wait or [])
        if len(waits) > 1:
            drain_inst.ins.sync_info.on_wait = waits[:1]
            for w in waits[1:]:
                nop = nc.sync.nop(nofuse=True, hint="tail_wait_split")
                if nop.ins.sync_info is None:
                    nop.ins.sync_info = mybir.SyncInfo(on_wait=[w], on_update=[])
                else:
                    nop.ins.sync_info.on_wait = [w]
        nc.all_engine_barrier()
        assert self.sems is not None
        popped = nc._tile_sem_poison_stack.pop()
        assert popped is self._sem_poison
        nc.clear_and_free_semaphores(list(self.sems.allocated().values()))
        nc.all_engine_barrier()

    tile_mod.TileContext._drain_and_barrier = _patched


_install_tile_patch()
from concourse.tile import TileContext  # noqa: E402


def install_profile_shim():
    """antenv.axon_hooks is missing from this image; recreate it so
    run_bass_kernel_spmd(trace=True) can capture NTFF profiles."""
    if "antenv.axon_hooks" in sys.modules:
        return
    import antenv

    mod = types.ModuleType("antenv.axon_hooks")
    mod._hook = None
    mod.set_axon_ntff_profile_hook = lambda h: setattr(mod, "_hook", h)
    mod.get_axon_ntff_profile_hook = lambda: mod._hook
    sys.modules["antenv.axon_hooks"] = mod
    antenv.axon_hooks = mod
    try:
        if "/root/.axon_site" not in sys.path:
            sys.path.insert(0, "/root/.axon_site")
        from trn_agent_boot.trn_boot import _ntff_profile_via_ctypes

        hook = _ntff_profile_via_ctypes("/opt/axon/libaxon_pjrt.so")
        mod.set_axon_ntff_profile_hook(hook)
    except Exception:
        pass


def _ap(t_ap, offset, dims):
    """Custom flat AP over a tile's underlying tensor."""
    return bass.AP(t_ap.tensor, offset, [list(d) for d in dims])


def _pt(t):
    """Physical partition pitch (elements) of a tile."""
    return t[:, :].ap[0][0]


def _split_excess_waits(nc, max_waits=1):
    """This walrus build rejects instructions carrying more than ~1 sync-wait.
    Move excess waits onto same-engine NoOps inserted just before."""
    n_split = 0
    for f in nc.m.functions:
        for bb in f.blocks:
            newlist = []
            for inst in bb.instructions:
                si = getattr(inst, "sync_info", None)
                if si is not None and si.on_wait and len(si.on_wait) > max_waits:
                    waits = list(si.on_wait)
                    for k, w in enumerate(waits[max_waits:]):
                        nop = mybir.InstNoOp(
                            name=f"{inst.name}_ws{k}",
                            engine=inst.engine,
                            bass_nofuse=True,
                            sync_info=mybir.SyncInfo(on_wait=[w], on_update=[]),
                        )
                        newlist.append(nop)
                        n_split += 1
                    si.on_wait = waits[:max_waits]
                newlist.append(inst)
            try:
                bb.instructions[:] = newlist
            except TypeError:
                bb.set_instructions(newlist)
    return n_split


def build_nc():
    nc = bass.Bass(target_bir_lowering=False)

    wbA_p = nc.declare_dram_parameter("wbA", [128, WBA_W], BF16, isOutput=False)
    wbB_p = nc.declare_dram_parameter("wbB", [128, WBB_W], BF16, isOutput=False)
    wbC_p = nc.declare_dram_parameter("wbC", [128, WBC_W], BF16, isOutput=False)
    cf_p = nc.declare_dram_parameter("cf", [128, CF_W], F32, isOutput=False)
    selm_p = nc.declare_dram_parameter("selm", [36, 9 * 128], BF16, isOutput=False)
    xin = nc.declare_dram_parameter("xin", [CIN, HW * HW], F32, isOutput=False)
    out = nc.declare_dram_parameter("out", [COUT, HW * HW], BF16, isOutput=True)

    with TileContext(nc) as tc:
        with (
            tc.tile_pool(name="sb", bufs=1) as sb,
            tc.tile_pool(name="sbx", bufs=1) as sbx,
            tc.tile_pool(name="sbo", bufs=2) as sbo,
            tc.tile_pool(name="ps", bufs=2, space="PSUM") as ps,
            tc.tile_pool(name="psc", bufs=4, space="PSUM") as psc,
            tc.tile_pool(name="dram", bufs=1, space="DRAM") as dram,
        ):
            # ================ SP-ring loads: packed weights/constants
            wbA = sb.tile([128, WBA_W], BF16, tag="wbA", name="wbA")
            nc.sync.dma_start(out=wbA[:, :], in_=wbA_p[:, :])
            wbB = sb.tile([128, WBB_W], BF16, tag="wbB", name="wbB")
            nc.sync.dma_start(out=wbB[:, :], in_=wbB_p[:, :])
            wbC = sb.tile([128, WBC_W], BF16, tag="wbC", name="wbC")
            nc.sync.dma_start(out=wbC[:, :], in_=wbC_p[:, :])
            cf = sb.tile([128, CF_W], F32, tag="cf", name="cf")
            nc.sync.dma_start(out=cf[:, :], in_=cf_p[:, :])
            selsb = sb.tile([36, 9 * 128], BF16, tag="selsb", name="selsb")
            nc.sync.dma_start(out=selsb[:, :], in_=selm_p[:, :])
            ptA, ptB, ptC, ptF = _pt(wbA), _pt(wbB), _pt(wbC), _pt(cf)

            def cfb(col, parts=128):
                return _ap(cf, col, [[ptF, parts], [1, 1]])

            # ================ ACT-ring loads: big contiguous input chunks
            xsb = [sbx.tile([128, HW * HW], F32, tag=f"xsb{ch}", name=f"xsb{ch}")
                   for ch in range(4)]
            for ch in range(4):
                nc.scalar.dma_start(
                    out=xsb[ch][:, :], in_=xin[ch * 128:(ch + 1) * 128, :]
                )

            # ================ stage A: layer-1 (h = lrelu(W1 s + b1))
            h = [sb.tile([128, N * NPOS], BF16, tag=f"h{ot}", name=f"h{ot}") for ot in range(4)]
            for ot in range(4):
                hp = ps.tile([128, N * NPOS], F32, tag="sA", name="sA")
                for it in range(4):
                    nc.tensor.matmul(
                        hp[:, :],
                        _ap(wbA, OFF_W1 + it * CIN + ot * 128, [[ptA, 128], [1, 128]]),
                        _ap(wbA, OFF_ST + it * 128, [[ptA, 128], [1, 128]]),
                        start=(it == 0),
                        stop=(it == 3),
                    )
                nc.scalar.activation(
                    h[ot][:, :], hp[:, :], mybir.ActivationFunctionType.Identity,
                    bias=cfb(CF_B1 + ot),
                )
                zt = sb.tile([128, N * NPOS], BF16, tag="zt", name="zt")
                nc.vector.tensor_scalar_mul(zt[:, :], h[ot][:, :], 0.01)
                nc.vector.tensor_max(h[ot][:, :], h[ot][:, :], zt[:, :])

            # ---------------- stage A: dw2 slice for all samples
            agin = dram.tile([AG_SZ], BF16)
            agout = dram.tile([AG_SZ], BF16)
            dppk = [sb.tile([128, 80], BF16, tag=f"dppk{o2}", name=f"dppk{o2}")
                    for o2 in range(2)]
            for o2 in range(2):
                dps = ps.tile([128, N * 9], F32, tag="sA", name="sA")
                for kt in range(16):
                    it, tap = kt // 4, kt % 4
                    di, dj = tap // 2, tap % 2
                    rhs = _ap(h[it], di * 4 + dj,
                              [[_pt(h[it]), 128], [NPOS, N], [4, 3], [1, 3]])
                    nc.tensor.matmul(
                        dps[:, :],
                        _ap(wbB, kt * OSL + o2 * 128, [[ptB, 128], [1, 128]]),
                        rhs,
                        start=(kt == 0),
                        stop=(kt == 15),
                    )
                # dw taps -> cols {n*10+0..8} of dppk
                nc.scalar.activation(
                    _ap(dppk[o2], 0, [[_pt(dppk[o2]), 128], [10, N], [1, 9]]),
                    _ap(dps, 0, [[_pt(dps), 128], [9, N], [1, 9]]),
                    mybir.ActivationFunctionType.Identity,
                    bias=cfb(CF_B2 + o2),
                )

            # ---------------- stage A: pooled-style path (pk / pb)
            sp = [sb.tile([128, N], BF16, tag=f"sp{i}", name=f"sp{i}") for i in range(4)]
            spf = sb.tile([128, N], F32, tag="spf", name="spf")
            for i in range(4):
                nc.vector.tensor_reduce(
                    spf[:, :],
                    _ap(wbA, OFF_ST + i * 128, [[ptA, 128], [NPOS, N], [1, NPOS]]),
                    axis=mybir.AxisListType.X,
                    op=mybir.AluOpType.add,
                )
                nc.vector.tensor_scalar_mul(sp[i][:, :], spf[:, :], 1.0 / NPOS)

            def layer1(off, bcol, tagp):
                acts = []
                for ot in range(4):
                    ap_ = ps.tile([128, N], F32, tag="sA", name="sA")
                    for it in range(4):
                        nc.tensor.matmul(
                            ap_[:, :],
                            _ap(wbC, off + it * CIN + ot * 128, [[ptC, 128], [1, 128]]),
                            sp[it][:, :],
                            start=(it == 0),
                            stop=(it == 3),
                        )
                    a = sb.tile([128, N], BF16, tag=f"{tagp}a{ot}", name=f"{tagp}a{ot}")
                    nc.scalar.activation(
                        a[:, :], ap_[:, :], mybir.ActivationFunctionType.Identity,
                        bias=cfb(bcol + ot),
                    )
                    zt2 = sb.tile([128, N], BF16, tag="zt2", name="zt2")
                    nc.vector.tensor_scalar_mul(zt2[:, :], a[:, :], 0.01)
                    nc.vector.tensor_max(a[:, :], a[:, :], zt2[:, :])
                    acts.append(a)
                return acts

            a1 = layer1(OFF_PK1, CF_BK1, "pk1")
            c1 = layer1(OFF_PB1, CF_BB1, "pb1")

            for o2 in range(2):
                pp = ps.tile([128, N], F32, tag="sA", name="sA")
                for it in range(4):
                    nc.tensor.matmul(
                        pp[:, :],
                        _ap(wbC, OFF_PK2 + it * PKSL + o2 * 128, [[ptC, 128], [1, 128]]),
                        a1[it][:, :],
                        start=(it == 0),
                        stop=(it == 3),
                    )
                # pk -> col {n*10+9} of dppk
                nc.scalar.activation(
                    _ap(dppk[o2], 9, [[_pt(dppk[o2]), 128], [10, N]]),
                    pp[:, :],
                    mybir.ActivationFunctionType.Identity,
                    bias=cfb(CF_BK2 + o2),
                )
                # agin[n*BLK + (o2*128+p)*10 + pos] = dppk[o2][p, n*10+pos]
                nc.sync.dma_start(
                    out=_ap(agin[:], o2 * 128 * 10, [[10, 128], [BLK, N], [1, 10]]),
                    in_=_ap(dppk[o2], 0, [[_pt(dppk[o2]), 128], [1, 80]]),
                )

            pbp = ps.tile([64, N], F32, tag="sA", name="sA")
            for it in range(4):
                nc.tensor.matmul(
                    pbp[:, :],
                    _ap(wbC, OFF_PB2 + it * PBSL, [[ptC, 128], [1, PBSL]]),
                    c1[it][:, :],
                    start=(it == 0),
                    stop=(it == 3),
                )
            pbc = sb.tile([64, 32], BF16, tag="pbc", name="pbc")
            nc.scalar.activation(
                pbc[:, 0:8], pbp[:, :], mybir.ActivationFunctionType.Identity,
                bias=cfb(CF_BB2, 64),
            )
            nc.sync.dma_start(
                out=_ap(agin[:], BPB, [[1, 64], [BLK, N]]),
                in_=_ap(pbc, 0, [[_pt(pbc), 64], [1, N]]),
            )

            # ================ pad + f32->bf16 convert (overlaps collective)
            # All pad work on the scalar (ACT) queue: a late xsb chunk never
            # blocks stage-B/C vector work or the conv PSUM drain.
            xb = [sbx.tile([128, 4384], BF16, tag=f"xb{ch}", name=f"xb{ch}")
                  for ch in range(4)]
            for ch in range(4):
                ptx = _pt(xb[ch])
                nc.scalar.activation(
                    _ap(xb[ch], HWP + 1, [[ptx, 128], [HWP, HW], [1, HW]]),
                    _ap(xsb[ch], 0, [[_pt(xsb[ch]), 128], [HW, HW], [1, HW]]),
                    mybir.ActivationFunctionType.Copy,
                )
                # reflect rows (row0 <- row2, row65 <- row63)
                nc.scalar.activation(
                    _ap(xb[ch], 1, [[ptx, 128], [1, HW]]),
                    _ap(xb[ch], 2 * HWP + 1, [[ptx, 128], [1, HW]]),
                    mybir.ActivationFunctionType.Copy,
                )
                nc.scalar.activation(
                    _ap(xb[ch], 65 * HWP + 1, [[ptx, 128], [1, HW]]),
                    _ap(xb[ch], 63 * HWP + 1, [[ptx, 128], [1, HW]]),
                    mybir.ActivationFunctionType.Copy,
                )
                # reflect cols (col0 <- col2, col65 <- col63)
                nc.scalar.activation(
                    _ap(xb[ch], 0, [[ptx, 128], [HWP, HWP]]),
                    _ap(xb[ch], 2, [[ptx, 128], [HWP, HWP]]),
                    mybir.ActivationFunctionType.Copy,
                )
                nc.scalar.activation(
                    _ap(xb[ch], 65, [[ptx, 128], [HWP, HWP]]),
                    _ap(xb[ch], 63, [[ptx, 128], [HWP, HWP]]),
                    mybir.ActivationFunctionType.Copy,
                )

            # ================ AllToAll: core c receives, from every rank r,
            # rank r's o-slice of sample c's dynamic weights.
            nc.gpsimd.collective_compute(
                "AllToAll",
                mybir.AluOpType.bypass,
                replica_groups=[list(range(N))],
                ins=[agin[:].opt()],
                outs=[agout[:].opt()],
            )

            # ================ stage B + stage C, chunk-pipelined
            # D[ch][q(part), m2*10+t(t<9) | m2*10+9=pk]
            D = [sb.tile([128, 40], BF16, tag=f"D{ch}", name=f"D{ch}") for ch in range(4)]
            PBb = [sb.tile([128, 1], BF16, tag=f"PBb{ch}", name=f"PBb{ch}") for ch in range(4)]
            for ch in range(4):
                ptD = _pt(D[ch])
                ptPB = _pt(PBb[ch])
                for half in range(2):
                    r = 2 * ch + half
                    nc.sync.dma_start(
                        out=_ap(D[ch], half * 64 * ptD, [[ptD, 64], [1, 40]]),
                        in_=_ap(agout[:], r * BLK, [[40, 64], [1, 40]]),
                    )
                    nc.sync.dma_start(
                        out=_ap(PBb[ch], half * 64 * ptPB, [[ptPB, 64], [1, 1]]),
                        in_=_ap(agout[:], r * BLK + BPB, [[1, 64], [1, 1]]),
                    )

            S = [sb.tile([128, 9 * 128], BF16, tag=f"S{ch}", name=f"S{ch}") for ch in range(4)]
            PBf = [sb.tile([128, 1], F32, tag=f"PBf{ch}", name=f"PBf{ch}") for ch in range(4)]

            def build_S(ch):
                ptD = _pt(D[ch])
                nc.vector.tensor_copy(PBf[ch][:, :], PBb[ch][:, :])
                PKf = sb.tile([128, 4], F32, tag="PKf", name="PKf")
                nc.vector.tensor_copy(PKf[:, :], _ap(D[ch], 9, [[ptD, 128], [10, 4]]))
                dp = ps.tile([128, 144], F32, tag="sA", name="sA")
                for m2 in range(4):
                    nc.tensor.matmul(
                        dp[:, m2 * 36:(m2 + 1) * 36],
                        _ap(wbC, OFF_PERM + m2 * 128, [[ptC, 128], [1, 128]]),
                        _ap(D[ch], 0, [[ptD, 128], [10, 4], [1, 9]]),
                        start=True,
                        stop=True,
                    )
                wef = sb.tile([128, 64], F32, tag="wef", name="wef")
                tmp = sb.tile([128, 36], F32, tag="weftmp", name="weftmp")
                nc.vector.tensor_scalar_mul(
                    wef[:, 0:36], dp[:, 0:36], PKf[:, 0:1]
                )
                for m2 in range(1, 4):
                    nc.vector.tensor_scalar_mul(
                        tmp[:, :], dp[:, m2 * 36:(m2 + 1) * 36], PKf[:, m2:m2 + 1]
                    )
                    nc.vector.tensor_add(wef[:, 0:36], wef[:, 0:36], tmp[:, :])
                # expand W_eff -> block-diag S via PE select-matmuls + mask
                tp = ps.tile([36, 128], F32, tag="sA", name="sA")
                nc.tensor.matmul(
                    tp[:, :], wef[:, 0:36],
                    _ap(cf, CF_IDENT, [[ptF, 128], [1, 128]]),
                    is_transpose=True, start=True, stop=True,
                )
                wefT = sb.tile([36, 128], BF16, tag="wefT", name="wefT")
                nc.vector.tensor_copy(wefT[:, :], tp[:, :])
                for t in range(9):
                    sps = ps.tile([128, 128], F32, tag="sB", name="sB")
                    nc.tensor.matmul(
                        sps[:, :],
                        selsb[:, t * 128:(t + 1) * 128],
                        wefT[:, :],
                        start=True, stop=True,
                    )
                    nc.vector.tensor_tensor(
                        S[ch][:, t * 128:(t + 1) * 128], sps[:, :],
                        _ap(cf, CF_MASK, [[ptF, 128], [1, 128]]),
                        op=mybir.AluOpType.mult,
                    )

            for ch in range(4):
                build_S(ch)
                ptx = _pt(xb[ch])
                osb = sbo.tile([128, HW * HW], BF16, tag="osb", name="osb")
                for sub in range(8):
                    cps = psc.tile([128, 512], F32, tag="cps", name="cps")
                    r0 = sub * 8
                    for tap in range(9):
                        di, dj = tap // 3, tap % 3
                        rhs = _ap(xb[ch], (r0 + di) * HWP + dj,
                                  [[ptx, 128], [HWP, 8], [1, HW]])
                        nc.tensor.matmul(
                            cps[:, :],
                            S[ch][:, tap * 128:(tap + 1) * 128],
                            rhs,
                            start=(tap == 0),
                            stop=(tap == 8),
                        )
                    nc.vector.tensor_scalar_add(
                        osb[:, r0 * HW:(r0 + 8) * HW], cps[:, :],
                        PBf[ch][:, 0:1],
                    )
                nc.scalar.dma_start(
                    out=out[ch * 128:(ch + 1) * 128, :], in_=osb[:, :]
                )

    _split_excess_waits(nc)
    return nc


_NC_CACHE = {}


def _get_nc():
    if "nc" not in _NC_CACHE:
        _NC_CACHE["nc"] = build_nc()
    return _NC_CACHE["nc"]


def make_in_maps(inputs):
    """Host-side shard/layout prep (pure layout: transpose/reshape/slice)."""
    style = np.asarray(inputs["style_encoding"], np.float32)
    pred = np.asarray(inputs["predicted"], np.float32)
    w1 = np.asarray(inputs["dw1_w"], np.float32).reshape(512, 512)
    w2 = np.asarray(inputs["dw2_w"], np.float32).reshape(2048, 512, 2, 2)
    pk1 = np.asarray(inputs["pk1_w"], np.float32).reshape(512, 512)
    pk2 = np.asarray(inputs["pk2_w"], np.float32).reshape(2048, 512)
    pb1 = np.asarray(inputs["pb1_w"], np.float32).reshape(512, 512)
    pb2 = np.asarray(inputs["pb2_w"], np.float32).reshape(512, 512)

    def blk128(mat_t):
        # [512, W] (row = input-ch) -> [128, 4*W] with block it at cols it*W
        W = mat_t.shape[1]
        return mat_t.reshape(4, 128, W).transpose(1, 0, 2).reshape(128, 4 * W)

    st_all = np.ascontiguousarray(
        style.transpose(1, 0, 2, 3).reshape(512, N * NPOS)
    )
    w1A = blk128(np.ascontiguousarray(w1.T))
    pk1A = blk128(np.ascontiguousarray(pk1.T))
    pb1A = blk128(np.ascontiguousarray(pb1.T))
    w2t_full = (
        w2.reshape(2048, 4, 128, 2, 2)
        .transpose(1, 3, 4, 2, 0)          # [it, di, dj, 128, o]
        .reshape(16, 128, 2048)
    )
    pk2t_full = np.ascontiguousarray(pk2.T).reshape(4, 128, 2048)
    pb2t_full = np.ascontiguousarray(pb2.T).reshape(4, 128, 512)

    permm = np.zeros((4, 128, 128), np.float32)
    for m2 in range(4):
        for p in range(128):
            permm[m2, 4 * (p // 4) + m2, p] = 1.0
    permA = permm.transpose(1, 0, 2).reshape(128, 512)
    identm = np.eye(128, dtype=np.float32)
    selm = np.zeros((36, 9, 128), np.float32)
    for t in range(9):
        for p in range(128):
            selm[(p % 4) * 9 + t, t, p] = 1.0
    selm = selm.reshape(36, 9 * 128).astype(BF16_NP)
    maskm = np.zeros((128, 128), np.float32)
    for p in range(128):
        for col in range(128):
            if p // 4 == col // 4:
                maskm[p, col] = 1.0

    wbA = np.hstack([
        st_all.reshape(4, 128, N * NPOS).transpose(1, 0, 2).reshape(128, 512),
        w1A,
    ]).astype(BF16_NP)
    b1c = np.asarray(inputs["dw1_b"], np.float32).reshape(4, 128).T
    bk1c = np.asarray(inputs["pk1_b"], np.float32).reshape(4, 128).T
    bb1c = np.asarray(inputs["pb1_b"], np.float32).reshape(4, 128).T

    in_maps = []
    for c in range(N):
        w2c = w2t_full[:, :, c * OSL:(c + 1) * OSL]       # [16,128,256]
        wbB = w2c.transpose(1, 0, 2).reshape(128, WBB_W).astype(BF16_NP)
        pk2c = pk2t_full[:, :, c * PKSL:(c + 1) * PKSL]   # [4,128,256]
        pb2c = pb2t_full[:, :, c * PBSL:(c + 1) * PBSL]   # [4,128,64]
        wbC = np.hstack([
            pk1A, pb1A,
            pk2c.transpose(1, 0, 2).reshape(128, 1024),
            pb2c.transpose(1, 0, 2).reshape(128, 256),
            permA,
        ]).astype(BF16_NP)
        b2c = np.asarray(inputs["dw2_b"], np.float32)[c * OSL:(c + 1) * OSL]
        bk2c = np.asarray(inputs["pk2_b"], np.float32)[c * PKSL:(c + 1) * PKSL]
        bb2c = np.asarray(inputs["pb2_b"], np.float32)[c * PBSL:(c + 1) * PBSL]
        bb2col = np.zeros((128, 1), np.float32)
        bb2col[:64, 0] = bb2c
        cf = np.hstack([
            b1c,
            b2c.reshape(2, 128).T,
            bk1c, bb1c,
            bk2c.reshape(2, 128).T,
            bb2col,
            identm, maskm,
        ]).astype(np.float32)
        assert cf.shape[1] == CF_W
        m = {
            "wbA": wbA,
            "wbB": np.ascontiguousarray(wbB),
            "wbC": np.ascontiguousarray(wbC),
            "cf": np.ascontiguousarray(cf),
            "selm": selm,
            "xin": np.ascontiguousarray(pred[c].reshape(512, HW * HW)),
        }
        in_maps.append(m)
    return in_maps


def kernel(**inputs):
    install_profile_shim()
    from concourse.bass_utils import run_bass_kernel_spmd

    nc = _get_nc()
    in_maps = make_in_maps(inputs)
    res = run_bass_kernel_spmd(nc, in_maps, core_ids=list(range(N)))
    outs = [np.asarray(res.results[c]["out"]).astype(np.float32).reshape(COUT, HW, HW)
            for c in range(N)]
    return np.stack(outs, axis=0)
